# revision 17
# baseline (speedup 1.0000x reference)
"""NoisyTopKRouter Trainium2 kernel.

Computes, for hidden_states [B,S,H], noise [B,S,E], w_gate/w_noise [E,H]:
    router_logits = X @ Wg^T + noise * softplus(X @ Wn^T)
    top2 of softmax(router_logits)  -> normalized dense gate weights + indices

Sharding: tokens (B*S = 16384) split evenly across 8 NeuronCores (2048
tokens/core); the small weights are replicated.  All math is token-local so
there are no collectives.

Layout strategy (per core):
  - Host pre-permutes the X shard into the exact k-chunk/token-strip stream
    order ([KG, STRIPS, 128, KPG, U]) so every input DMA reads a fully
    contiguous region with 16KB-contiguous per-partition runs.
  - Matmul: stationary = xt chunk [128h x 128t], moving = packed weights
    [128h x NW] (cols 0:64 gate, 64:128 noise) -> PSUM [128t x NW] token-major.
    Accumulate over the 32 k-chunks of H.  One PSUM bank per 128-token tile
    (start=True clears has_written for the whole bank).
  - Epilogue per 512-token strip: softplus via relu(x)+ln(1+exp(-|x|)) (ACT
    exp/ln share one table set), max8/max_index (DVE) for top-2 values +
    indices, top-2 softmax weights from 1/(1+exp(l2-l1)), dense scatter via
    is_equal masks.
  - Outputs are written in the on-chip [STRIPS, 128, M_TILES, *] layout
    (contiguous DMA); the host inverse-permutes to the natural token order.
"""

import os
import numpy as np

import concourse.bass as bass
import concourse.bacc as bacc
import concourse.mybir as mybir
from concourse import tile
from concourse.bass_utils import run_bass_kernel_spmd

B, S, H, E = 4, 4096, 4096, 64
NCORES = 8
T = B * S               # 16384 tokens total
TC = T // NCORES        # 2048 tokens per core
STRIPS = 4              # strips per core
U = TC // STRIPS        # 512 tokens per strip
M_TILES = U // 128      # 4 x 128-token tiles per strip
KB = H // 128           # 32 contraction chunks
KG = 2                  # k-groups (DMA batches) per strip
KPG = KB // KG          # 16 k-chunks per DMA batch (4MB per DMA)

F32 = mybir.dt.float32
U32 = mybir.dt.uint32

_BUILD_CACHE = {}

# results of the last run (BassKernelResults); test.py reads exec_time_ns
LAST_RESULTS = None


def _build(mm_dtype, nw):
    """Build the SPMD Bass program. mm_dtype: matmul input dtype
    (float32 or float32r); nw: moving-operand width (>=128; cols >=128 are
    zero padding used to hit the fast fp32r streaming rate)."""
    nc = bacc.Bacc()

    xt = nc.dram_tensor("xt", [KG, STRIPS, 128, KPG, U], mm_dtype,
                        kind="ExternalInput")
    nz = nc.dram_tensor("nz", [STRIPS, 128, M_TILES, E], F32,
                        kind="ExternalInput")
    wt = nc.dram_tensor("wt", [H, nw], mm_dtype, kind="ExternalInput")
    ident = nc.dram_tensor("ident", [128, 128], F32, kind="ExternalInput")
    logits_o = nc.dram_tensor("logits_o", [STRIPS, 128, M_TILES, E], F32,
                              kind="ExternalOutput")
    gates_o = nc.dram_tensor("gates_o", [STRIPS, 128, M_TILES, E], F32,
                             kind="ExternalOutput")
    idx_o = nc.dram_tensor("idx_o", [STRIPS, 128, M_TILES, 2], U32,
                           kind="ExternalOutput")

    wt_v = wt[:, :].rearrange("(k p) e -> p k e", p=128)

    AF = mybir.ActivationFunctionType
    OP = mybir.AluOpType

    with tile.TileContext(nc) as tc:
        with (
            tc.tile_pool(name="const", bufs=1) as cpool,
            tc.tile_pool(name="xpool", bufs=4) as xpool,
            tc.tile_pool(name="opool", bufs=2) as opool,
            tc.tile_pool(name="spool", bufs=2) as spool,
            tc.tile_pool(name="psum", bufs=2, space=bass.MemorySpace.PSUM) as ppool,
        ):
            wt_sb = cpool.tile([128, KB, nw], mm_dtype, tag="wt")
            nc.sync.dma_start(out=wt_sb[:], in_=wt_v)
            id_sb = cpool.tile([128, 128], F32, tag="id")
            nc.sync.dma_start(out=id_sb[:], in_=ident[:, :])

            for s in range(STRIPS):
                # W-stationary matmul: out = L^T [128 (gate|noise), U tokens],
                # moving operand is the 512-token x^T chunk (wide moving dim
                # amortizes the fp32 2-pass streaming cost per instruction).
                psL = ppool.tile([128, U], F32, tag="psL", name="psL")
                for kg in range(KG):
                    xb = xpool.tile([128, KPG, U], mm_dtype, tag="xb")
                    # Ping-pong the two HWDGE rings (SP / ACT) so one ring's
                    # completion tail overlaps the other's transfer.
                    eng = nc.sync if (s * KG + kg) % 2 == 0 else nc.scalar
                    eng.dma_start(out=xb[:], in_=xt[kg, s])
                    for kk in range(KPG):
                        k = kg * KPG + kk
                        nc.tensor.matmul(
                            psL[:, :],
                            wt_sb[:, k, 0:128],
                            xb[:, kk, :],
                            start=(k == 0),
                            stop=(k == KB - 1),
                        )

                # ---- epilogue for this strip (512 tokens) ----
                nzt = spool.tile([128, M_TILES, E], F32, tag="nzt")
                nc.gpsimd.dma_start(out=nzt[:], in_=nz[s])

                lgt = opool.tile([128, M_TILES, E], F32, tag="lgt")
                gat = opool.tile([128, M_TILES, E], F32, tag="gat")
                ixt = opool.tile([128, M_TILES, 2], U32, tag="ixt")

                # Transpose L^T -> token-major [128 tokens, gate|noise] via PE
                # (PSUM -> SBUF copy first: PE cannot read PSUM).
                lT = opool.tile([128, U], F32, tag="lT")
                nc.vector.tensor_copy(lT[:], psL[:, :])
                psT = ppool.tile([128, U], F32, tag="psT", name="psT")
                for m in range(M_TILES):
                    nc.tensor.transpose(
                        psT[:, m * 128:(m + 1) * 128],
                        lT[:, m * 128:(m + 1) * 128],
                        id_sb[:],
                    )
                psv = psT[:].rearrange("p (m e) -> p m e", e=128)
                gl = psv[:, :, 0:E]       # gate logits [128, M, E] (PSUM)
                nl = psv[:, :, E:2 * E]   # noise logits [128, M, E] (PSUM)

                # softplus(x) = relu(x) + ln(1 + exp(-|x|))  (no Softplus ACT
                # table in this toolchain; exp/ln share one table set).
                # Batched across the whole strip to limit ACT table swaps.
                ab = spool.tile([128, M_TILES, E], F32, tag="ab")
                nc.scalar.activation(ab[:], nl, AF.Abs)
                ex = spool.tile([128, M_TILES, E], F32, tag="ex")
                nc.scalar.activation(ex[:], ab[:], AF.Exp, scale=-1.0)
                ln1 = spool.tile([128, M_TILES, E], F32, tag="ln1")
                nc.scalar.activation(ln1[:], ex[:], AF.Ln, bias=1.0)
                sp_t = spool.tile([128, M_TILES, E], F32, tag="sp")
                nc.vector.scalar_tensor_tensor(
                    sp_t[:], nl, 0.0, ln1[:], OP.max, OP.add
                )
                tmp = spool.tile([128, M_TILES, E], F32, tag="tmp")
                nc.vector.tensor_mul(tmp[:], nzt[:], sp_t[:])
                nc.vector.tensor_add(lgt[:], gl, tmp[:])

                mxa = spool.tile([128, M_TILES, 8], F32, tag="mxa")
                ixa = spool.tile([128, M_TILES, 8], U32, tag="ixa")
                for m in range(M_TILES):
                    nc.vector.max(mxa[:, m, :], lgt[:, m, :])
                    nc.vector.max_index(ixa[:, m, :], mxa[:, m, :], lgt[:, m, :])

                # top-2 softmax weights: w1 = 1/(1+exp(l2-l1)), w2 = 1-w1
                d = spool.tile([128, M_TILES], F32, tag="d")
                nc.vector.tensor_sub(d[:], mxa[:, :, 1], mxa[:, :, 0])
                e2 = spool.tile([128, M_TILES], F32, tag="e2")
                nc.scalar.activation(e2[:], d[:], AF.Exp)
                opd = spool.tile([128, M_TILES], F32, tag="opd")
                nc.vector.tensor_scalar_add(opd[:], e2[:], 1.0)
                w1 = spool.tile([128, M_TILES], F32, tag="w1")
                nc.vector.reciprocal(w1[:], opd[:])
                w2 = spool.tile([128, M_TILES], F32, tag="w2")
                nc.vector.tensor_mul(w2[:], e2[:], w1[:])

                for m in range(M_TILES):
                    lg = lgt[:, m, :]
                    eq1 = spool.tile([128, E], F32, tag="eq1")
                    nc.vector.tensor_scalar(
                        eq1[:], lg, mxa[:, m, 0:1], None, OP.is_equal
                    )
                    eq2 = spool.tile([128, E], F32, tag="eq2")
                    nc.vector.tensor_scalar(
                        eq2[:], lg, mxa[:, m, 1:2], None, OP.is_equal
                    )
                    t2 = spool.tile([128, E], F32, tag="t2")
                    nc.vector.tensor_scalar_mul(t2[:], eq2[:], w2[:, m:m + 1])
                    nc.vector.scalar_tensor_tensor(
                        gat[:, m, :], eq1[:], w1[:, m:m + 1], t2[:],
                        OP.mult, OP.add,
                    )
                nc.vector.tensor_copy(ixt[:], ixa[:, :, 0:2])

                nc.gpsimd.dma_start(out=logits_o[s], in_=lgt[:])
                nc.gpsimd.dma_start(out=gates_o[s], in_=gat[:])
                nc.gpsimd.dma_start(out=idx_o[s], in_=ixt[:])

    nc.compile()
    return nc


def kernel(hidden_states, noise, w_gate, w_noise, mm_dtype="fp32", trace=None):
    global LAST_RESULTS

    if trace is None:
        trace = bool(os.environ.get("KERNEL_TRACE"))
    mm_dt = {"fp32": F32, "fp32r": mybir.dt.float32r}[mm_dtype]
    # fp32r streams at full rate only with a moving dim >= 256 -> zero-pad.
    nw = 128 if mm_dtype == "fp32" else 256

    key = (mm_dtype, nw)
    if key not in _BUILD_CACHE:
        _BUILD_CACHE[key] = _build(mm_dt, nw)
    nc = _BUILD_CACHE[key]

    X = np.ascontiguousarray(np.asarray(hidden_states, dtype=np.float32)).reshape(T, H)
    nzf = np.ascontiguousarray(np.asarray(noise, dtype=np.float32)).reshape(T, E)
    wg = np.asarray(w_gate, dtype=np.float32)
    wn = np.asarray(w_noise, dtype=np.float32)

    wt_full = np.zeros((H, nw), dtype=np.float32)
    wt_full[:, :E] = wg.T
    wt_full[:, E:2 * E] = wn.T

    # xt stream order: [KG, STRIPS, 128p, KPG, U] from X^T [H, TC]
    # X^T[h, t] with h = (kg*KPG + kk)*128 + p, t = s*U + u.
    xt_all = (
        X.reshape(NCORES, TC, H)
        .transpose(0, 2, 1)                          # [c, H, TC]
        .reshape(NCORES, KG, KPG, 128, STRIPS, U)
        .transpose(0, 1, 4, 3, 2, 5)                 # [c, KG, STRIPS, p, KPG, U]
    )
    xt_all = np.ascontiguousarray(xt_all)
    # noise order: [STRIPS, 128p, M_TILES, E]; token t = s*U + m*128 + p
    nz_all = np.ascontiguousarray(
        nzf.reshape(NCORES, STRIPS, M_TILES, 128, E).transpose(0, 1, 3, 2, 4)
    )

    ident = np.eye(128, dtype=np.float32)
    in_maps = [
        {"xt": xt_all[c], "nz": nz_all[c], "wt": wt_full, "ident": ident}
        for c in range(NCORES)
    ]

    LAST_RESULTS = run_bass_kernel_spmd(
        nc, in_maps, list(range(NCORES)), trace=trace
    )
    results = LAST_RESULTS.results

    def unshuffle(key, dt):
        # [c][STRIPS, 128, M_TILES, X] -> token order t = s*U + m*128 + p
        a = np.stack([results[c][key] for c in range(NCORES)])
        a = a.transpose(0, 1, 3, 2, 4)  # [c, s, m, p, X]
        return np.ascontiguousarray(a).reshape(T, -1).astype(dt, copy=False)

    logits = unshuffle("logits_o", np.float32).reshape(B, S, E)
    gates = unshuffle("gates_o", np.float32).reshape(B, S, E)
    idx = unshuffle("idx_o", np.int32).reshape(B, S, 2)
    return gates, idx, logits


# revision 20
# speedup vs baseline: 1.0488x; 1.0488x over previous
"""NoisyTopKRouter Trainium2 kernel.

Computes, for hidden_states [B,S,H], noise [B,S,E], w_gate/w_noise [E,H]:
    router_logits = X @ Wg^T + noise * softplus(X @ Wn^T)
    top2 of softmax(router_logits)  -> normalized dense gate weights + indices

Sharding: tokens (B*S = 16384) split evenly across 8 NeuronCores (2048
tokens/core); the small weights are replicated.  All math is token-local so
there are no collectives.

Layout strategy (per core):
  - Host pre-permutes the X shard into the exact k-chunk/token-strip stream
    order ([KG, STRIPS, 128, KPG, U]) so every input DMA reads a fully
    contiguous region with 16KB-contiguous per-partition runs.
  - Matmul: stationary = xt chunk [128h x 128t], moving = packed weights
    [128h x NW] (cols 0:64 gate, 64:128 noise) -> PSUM [128t x NW] token-major.
    Accumulate over the 32 k-chunks of H.  One PSUM bank per 128-token tile
    (start=True clears has_written for the whole bank).
  - Epilogue per 512-token strip: softplus via relu(x)+ln(1+exp(-|x|)) (ACT
    exp/ln share one table set), max8/max_index (DVE) for top-2 values +
    indices, top-2 softmax weights from 1/(1+exp(l2-l1)), dense scatter via
    is_equal masks.
  - Outputs are written in the on-chip [STRIPS, 128, M_TILES, *] layout
    (contiguous DMA); the host inverse-permutes to the natural token order.
"""

import os
import numpy as np

import concourse.bass as bass
import concourse.bacc as bacc
import concourse.mybir as mybir
from concourse import tile
from concourse.bass_utils import run_bass_kernel_spmd

B, S, H, E = 4, 4096, 4096, 64
NCORES = 8
T = B * S               # 16384 tokens total
TC = T // NCORES        # 2048 tokens per core
STRIPS = 8              # strips per core
U = TC // STRIPS        # 256 tokens per strip
M_TILES = U // 128      # 2 x 128-token tiles per strip
KB = H // 128           # 32 contraction chunks
KG = 1                  # k-groups (DMA batches) per strip
KPG = KB // KG          # 32 k-chunks per DMA batch (4MB per DMA)

F32 = mybir.dt.float32
U32 = mybir.dt.uint32

_BUILD_CACHE = {}

# results of the last run (BassKernelResults); test.py reads exec_time_ns
LAST_RESULTS = None


def _build(mm_dtype, nw):
    """Build the SPMD Bass program. mm_dtype: matmul input dtype
    (float32 or float32r); nw: moving-operand width (>=128; cols >=128 are
    zero padding used to hit the fast fp32r streaming rate)."""
    nc = bacc.Bacc()

    xt = nc.dram_tensor("xt", [KG, STRIPS, 128, KPG, U], mm_dtype,
                        kind="ExternalInput")
    nz = nc.dram_tensor("nz", [STRIPS, 128, M_TILES, E], F32,
                        kind="ExternalInput")
    wt = nc.dram_tensor("wt", [H, nw], mm_dtype, kind="ExternalInput")
    ident = nc.dram_tensor("ident", [128, 128], F32, kind="ExternalInput")
    logits_o = nc.dram_tensor("logits_o", [STRIPS, 128, M_TILES, E], F32,
                              kind="ExternalOutput")
    gates_o = nc.dram_tensor("gates_o", [STRIPS, 128, M_TILES, E], F32,
                             kind="ExternalOutput")
    idx_o = nc.dram_tensor("idx_o", [STRIPS, 128, M_TILES, 2], U32,
                           kind="ExternalOutput")

    wt_v = wt[:, :].rearrange("(k p) e -> p k e", p=128)

    AF = mybir.ActivationFunctionType
    OP = mybir.AluOpType

    with tile.TileContext(nc) as tc:
        with (
            tc.tile_pool(name="const", bufs=1) as cpool,
            tc.tile_pool(name="xpool", bufs=4) as xpool,
            tc.tile_pool(name="opool", bufs=2) as opool,
            tc.tile_pool(name="spool", bufs=2) as spool,
            tc.tile_pool(name="psum", bufs=2, space=bass.MemorySpace.PSUM) as ppool,
        ):
            # Weights split so the first matmul's slice lands fast, and the
            # sync ring carries them while the scalar ring streams strip 0.
            wt_sb = cpool.tile([128, KB, nw], mm_dtype, tag="wt")
            nc.sync.dma_start(out=wt_sb[:, 0:4, :], in_=wt_v[:, 0:4, :])
            id_sb = cpool.tile([128, 128], F32, tag="id")
            nc.sync.dma_start(out=id_sb[:], in_=ident[:, :])
            nc.sync.dma_start(out=wt_sb[:, 4:KB, :], in_=wt_v[:, 4:KB, :])

            for s in range(STRIPS):
                # W-stationary matmul: out = L^T [128 (gate|noise), U tokens],
                # moving operand is the 512-token x^T chunk (wide moving dim
                # amortizes the fp32 2-pass streaming cost per instruction).
                psL = ppool.tile([128, U], F32, tag="psL", name="psL")
                for kg in range(KG):
                    xb = xpool.tile([128, KPG, U], mm_dtype, tag="xb")
                    # Ping-pong the two HWDGE rings (SP / ACT) so one ring's
                    # completion tail overlaps the other's transfer.  Strip 0
                    # is split into 1MB pieces so the first matmul starts
                    # ~10x earlier than a monolithic 4MB load would allow.
                    eng = nc.scalar if s % 2 == 0 else nc.sync
                    if s == 0:
                        for j in range(4):
                            q = KPG // 4
                            eng.dma_start(
                                out=xb[:, j * q:(j + 1) * q, :],
                                in_=xt[kg, s, :, j * q:(j + 1) * q, :],
                            )
                    else:
                        eng.dma_start(out=xb[:], in_=xt[kg, s])
                    for kk in range(KPG):
                        k = kg * KPG + kk
                        nc.tensor.matmul(
                            psL[:, :],
                            wt_sb[:, k, 0:128],
                            xb[:, kk, :],
                            start=(k == 0),
                            stop=(k == KB - 1),
                        )

                # ---- epilogue for this strip (512 tokens) ----
                nzt = spool.tile([128, M_TILES, E], F32, tag="nzt")
                nc.gpsimd.dma_start(out=nzt[:], in_=nz[s])

                lgt = opool.tile([128, M_TILES, E], F32, tag="lgt")
                gat = opool.tile([128, M_TILES, E], F32, tag="gat")
                ixt = opool.tile([128, M_TILES, 2], U32, tag="ixt")

                # Transpose L^T -> token-major [128 tokens, gate|noise] via PE
                # (PSUM -> SBUF copy first: PE cannot read PSUM).
                lT = opool.tile([128, U], F32, tag="lT")
                nc.vector.tensor_copy(lT[:], psL[:, :])
                psT = ppool.tile([128, U], F32, tag="psT", name="psT")
                for m in range(M_TILES):
                    nc.tensor.transpose(
                        psT[:, m * 128:(m + 1) * 128],
                        lT[:, m * 128:(m + 1) * 128],
                        id_sb[:],
                    )
                psv = psT[:].rearrange("p (m e) -> p m e", e=128)
                gl = psv[:, :, 0:E]       # gate logits [128, M, E] (PSUM)
                nl = psv[:, :, E:2 * E]   # noise logits [128, M, E] (PSUM)

                # softplus(x) = relu(x) + ln(1 + exp(-|x|))  (no Softplus ACT
                # table in this toolchain; exp/ln share one table set).
                # Batched across the whole strip to limit ACT table swaps.
                ab = spool.tile([128, M_TILES, E], F32, tag="ab")
                nc.scalar.activation(ab[:], nl, AF.Abs)
                ex = spool.tile([128, M_TILES, E], F32, tag="ex")
                nc.scalar.activation(ex[:], ab[:], AF.Exp, scale=-1.0)
                ln1 = spool.tile([128, M_TILES, E], F32, tag="ln1")
                nc.scalar.activation(ln1[:], ex[:], AF.Ln, bias=1.0)
                sp_t = spool.tile([128, M_TILES, E], F32, tag="sp")
                nc.vector.scalar_tensor_tensor(
                    sp_t[:], nl, 0.0, ln1[:], OP.max, OP.add
                )
                tmp = spool.tile([128, M_TILES, E], F32, tag="tmp")
                nc.vector.tensor_mul(tmp[:], nzt[:], sp_t[:])
                nc.vector.tensor_add(lgt[:], gl, tmp[:])

                mxa = spool.tile([128, M_TILES, 8], F32, tag="mxa")
                ixa = spool.tile([128, M_TILES, 8], U32, tag="ixa")
                for m in range(M_TILES):
                    nc.vector.max(mxa[:, m, :], lgt[:, m, :])
                    nc.vector.max_index(ixa[:, m, :], mxa[:, m, :], lgt[:, m, :])

                # top-2 softmax weights: w1 = 1/(1+exp(l2-l1)), w2 = 1-w1
                d = spool.tile([128, M_TILES], F32, tag="d")
                nc.vector.tensor_sub(d[:], mxa[:, :, 1], mxa[:, :, 0])
                e2 = spool.tile([128, M_TILES], F32, tag="e2")
                nc.scalar.activation(e2[:], d[:], AF.Exp)
                opd = spool.tile([128, M_TILES], F32, tag="opd")
                nc.vector.tensor_scalar_add(opd[:], e2[:], 1.0)
                w1 = spool.tile([128, M_TILES], F32, tag="w1")
                nc.vector.reciprocal(w1[:], opd[:])
                w2 = spool.tile([128, M_TILES], F32, tag="w2")
                nc.vector.tensor_mul(w2[:], e2[:], w1[:])

                for m in range(M_TILES):
                    lg = lgt[:, m, :]
                    eq1 = spool.tile([128, E], F32, tag="eq1")
                    nc.vector.tensor_scalar(
                        eq1[:], lg, mxa[:, m, 0:1], None, OP.is_equal
                    )
                    eq2 = spool.tile([128, E], F32, tag="eq2")
                    nc.vector.tensor_scalar(
                        eq2[:], lg, mxa[:, m, 1:2], None, OP.is_equal
                    )
                    t2 = spool.tile([128, E], F32, tag="t2")
                    nc.vector.tensor_scalar_mul(t2[:], eq2[:], w2[:, m:m + 1])
                    nc.vector.scalar_tensor_tensor(
                        gat[:, m, :], eq1[:], w1[:, m:m + 1], t2[:],
                        OP.mult, OP.add,
                    )
                nc.vector.tensor_copy(ixt[:], ixa[:, :, 0:2])

                nc.gpsimd.dma_start(out=logits_o[s], in_=lgt[:])
                nc.gpsimd.dma_start(out=gates_o[s], in_=gat[:])
                nc.gpsimd.dma_start(out=idx_o[s], in_=ixt[:])

    nc.compile()
    return nc


def kernel(hidden_states, noise, w_gate, w_noise, mm_dtype="fp32", trace=None):
    global LAST_RESULTS

    if trace is None:
        trace = bool(os.environ.get("KERNEL_TRACE"))
    mm_dt = {"fp32": F32, "fp32r": mybir.dt.float32r}[mm_dtype]
    # fp32r streams at full rate only with a moving dim >= 256 -> zero-pad.
    nw = 128 if mm_dtype == "fp32" else 256

    key = (mm_dtype, nw)
    if key not in _BUILD_CACHE:
        _BUILD_CACHE[key] = _build(mm_dt, nw)
    nc = _BUILD_CACHE[key]

    X = np.ascontiguousarray(np.asarray(hidden_states, dtype=np.float32)).reshape(T, H)
    nzf = np.ascontiguousarray(np.asarray(noise, dtype=np.float32)).reshape(T, E)
    wg = np.asarray(w_gate, dtype=np.float32)
    wn = np.asarray(w_noise, dtype=np.float32)

    wt_full = np.zeros((H, nw), dtype=np.float32)
    wt_full[:, :E] = wg.T
    wt_full[:, E:2 * E] = wn.T

    # xt stream order: [KG, STRIPS, 128p, KPG, U] from X^T [H, TC]
    # X^T[h, t] with h = (kg*KPG + kk)*128 + p, t = s*U + u.
    xt_all = (
        X.reshape(NCORES, TC, H)
        .transpose(0, 2, 1)                          # [c, H, TC]
        .reshape(NCORES, KG, KPG, 128, STRIPS, U)
        .transpose(0, 1, 4, 3, 2, 5)                 # [c, KG, STRIPS, p, KPG, U]
    )
    xt_all = np.ascontiguousarray(xt_all)
    # noise order: [STRIPS, 128p, M_TILES, E]; token t = s*U + m*128 + p
    nz_all = np.ascontiguousarray(
        nzf.reshape(NCORES, STRIPS, M_TILES, 128, E).transpose(0, 1, 3, 2, 4)
    )

    ident = np.eye(128, dtype=np.float32)
    in_maps = [
        {"xt": xt_all[c], "nz": nz_all[c], "wt": wt_full, "ident": ident}
        for c in range(NCORES)
    ]

    LAST_RESULTS = run_bass_kernel_spmd(
        nc, in_maps, list(range(NCORES)), trace=trace
    )
    results = LAST_RESULTS.results

    def unshuffle(key, dt):
        # [c][STRIPS, 128, M_TILES, X] -> token order t = s*U + m*128 + p
        a = np.stack([results[c][key] for c in range(NCORES)])
        a = a.transpose(0, 1, 3, 2, 4)  # [c, s, m, p, X]
        return np.ascontiguousarray(a).reshape(T, -1).astype(dt, copy=False)

    logits = unshuffle("logits_o", np.float32).reshape(B, S, E)
    gates = unshuffle("gates_o", np.float32).reshape(B, S, E)
    idx = unshuffle("idx_o", np.int32).reshape(B, S, 2)
    return gates, idx, logits


# revision 22
# speedup vs baseline: 1.0857x; 1.0352x over previous
"""NoisyTopKRouter Trainium2 kernel.

Computes, for hidden_states [B,S,H], noise [B,S,E], w_gate/w_noise [E,H]:
    router_logits = X @ Wg^T + noise * softplus(X @ Wn^T)
    top2 of softmax(router_logits)  -> normalized dense gate weights + indices

Sharding: tokens (B*S = 16384) split evenly across 8 NeuronCores (2048
tokens/core); the small weights are replicated.  All math is token-local so
there are no collectives.

Layout strategy (per core):
  - Host pre-permutes the X shard into the exact k-chunk/token-strip stream
    order ([KG, STRIPS, 128, KPG, U]) so every input DMA reads a fully
    contiguous region with 16KB-contiguous per-partition runs.
  - Matmul: stationary = xt chunk [128h x 128t], moving = packed weights
    [128h x NW] (cols 0:64 gate, 64:128 noise) -> PSUM [128t x NW] token-major.
    Accumulate over the 32 k-chunks of H.  One PSUM bank per 128-token tile
    (start=True clears has_written for the whole bank).
  - Epilogue per 512-token strip: softplus via relu(x)+ln(1+exp(-|x|)) (ACT
    exp/ln share one table set), max8/max_index (DVE) for top-2 values +
    indices, top-2 softmax weights from 1/(1+exp(l2-l1)), dense scatter via
    is_equal masks.
  - Outputs are written in the on-chip [STRIPS, 128, M_TILES, *] layout
    (contiguous DMA); the host inverse-permutes to the natural token order.
"""

import os
import numpy as np

import concourse.bass as bass
import concourse.bacc as bacc
import concourse.mybir as mybir
from concourse import tile
from concourse.bass_utils import run_bass_kernel_spmd

B, S, H, E = 4, 4096, 4096, 64
NCORES = 8
T = B * S               # 16384 tokens total
TC = T // NCORES        # 2048 tokens per core
STRIPS = 4              # strips per core
U = TC // STRIPS        # 512 tokens per strip (N=512 hides fp32 LDWEIGHTS)
M_TILES = U // 128      # 4 x 128-token tiles per strip
KB = H // 128           # 32 contraction chunks
KG = 2                  # k-groups (DMA batches) per strip
KPG = KB // KG          # 16 k-chunks per DMA batch (4MB per DMA)

F32 = mybir.dt.float32
U32 = mybir.dt.uint32

_BUILD_CACHE = {}

# results of the last run (BassKernelResults); test.py reads exec_time_ns
LAST_RESULTS = None


def _build(mm_dtype, nw):
    """Build the SPMD Bass program. mm_dtype: matmul input dtype
    (float32 or float32r); nw: moving-operand width (>=128; cols >=128 are
    zero padding used to hit the fast fp32r streaming rate)."""
    nc = bacc.Bacc()

    xt = nc.dram_tensor("xt", [KG, STRIPS, 128, KPG, U], mm_dtype,
                        kind="ExternalInput")
    nz = nc.dram_tensor("nz", [STRIPS, 128, M_TILES, E], F32,
                        kind="ExternalInput")
    wt = nc.dram_tensor("wt", [H, nw], mm_dtype, kind="ExternalInput")
    ident = nc.dram_tensor("ident", [128, 128], F32, kind="ExternalInput")
    logits_o = nc.dram_tensor("logits_o", [STRIPS, 128, M_TILES, E], F32,
                              kind="ExternalOutput")
    gates_o = nc.dram_tensor("gates_o", [STRIPS, 128, M_TILES, E], F32,
                             kind="ExternalOutput")
    idx_o = nc.dram_tensor("idx_o", [STRIPS, 128, M_TILES, 2], U32,
                           kind="ExternalOutput")

    wt_v = wt[:, :].rearrange("(k p) e -> p k e", p=128)

    AF = mybir.ActivationFunctionType
    OP = mybir.AluOpType

    with tile.TileContext(nc) as tc:
        with (
            tc.tile_pool(name="const", bufs=1) as cpool,
            tc.tile_pool(name="xpool", bufs=4) as xpool,
            tc.tile_pool(name="opool", bufs=2) as opool,
            tc.tile_pool(name="spool", bufs=2) as spool,
            tc.tile_pool(name="psum", bufs=2, space=bass.MemorySpace.PSUM) as ppool,
        ):
            # Weights split so the first matmul's slice lands fast, and the
            # sync ring carries them while the scalar ring streams strip 0.
            wt_sb = cpool.tile([128, KB, nw], mm_dtype, tag="wt")
            nc.sync.dma_start(out=wt_sb[:, 0:4, :], in_=wt_v[:, 0:4, :])
            id_sb = cpool.tile([128, 128], F32, tag="id")
            nc.sync.dma_start(out=id_sb[:], in_=ident[:, :])
            nc.sync.dma_start(out=wt_sb[:, 4:KB, :], in_=wt_v[:, 4:KB, :])

            for s in range(STRIPS):
                # W-stationary matmul: out = L^T [128 (gate|noise), U tokens],
                # moving operand is the 512-token x^T chunk (wide moving dim
                # amortizes the fp32 2-pass streaming cost per instruction).
                psL = ppool.tile([128, U], F32, tag="psL", name="psL")
                for kg in range(KG):
                    xb = xpool.tile([128, KPG, U], mm_dtype, tag="xb")
                    # Ping-pong the two HWDGE rings (SP / ACT) so one ring's
                    # completion tail overlaps the other's transfer.  Strip 0
                    # is split into 1MB pieces so the first matmul starts
                    # ~10x earlier than a monolithic 4MB load would allow.
                    eng = nc.scalar if (s * KG + kg) % 2 == 0 else nc.sync
                    if s == 0 and kg == 0:
                        for j in range(4):
                            q = KPG // 4
                            eng.dma_start(
                                out=xb[:, j * q:(j + 1) * q, :],
                                in_=xt[kg, s, :, j * q:(j + 1) * q, :],
                            )
                    else:
                        eng.dma_start(out=xb[:], in_=xt[kg, s])
                    for kk in range(KPG):
                        k = kg * KPG + kk
                        nc.tensor.matmul(
                            psL[:, :],
                            wt_sb[:, k, 0:128],
                            xb[:, kk, :],
                            start=(k == 0),
                            stop=(k == KB - 1),
                        )

                # ---- epilogue for this strip (512 tokens) ----
                nzt = spool.tile([128, M_TILES, E], F32, tag="nzt")
                nc.gpsimd.dma_start(out=nzt[:], in_=nz[s])

                lgt = opool.tile([128, M_TILES, E], F32, tag="lgt")
                gat = opool.tile([128, M_TILES, E], F32, tag="gat")
                ixt = opool.tile([128, M_TILES, 2], U32, tag="ixt")

                # Transpose L^T -> token-major [128 tokens, gate|noise] via PE
                # (PSUM -> SBUF copy first: PE cannot read PSUM).
                lT = opool.tile([128, U], F32, tag="lT")
                nc.vector.tensor_copy(lT[:], psL[:, :])
                psT = ppool.tile([128, U], F32, tag="psT", name="psT")
                for m in range(M_TILES):
                    nc.tensor.transpose(
                        psT[:, m * 128:(m + 1) * 128],
                        lT[:, m * 128:(m + 1) * 128],
                        id_sb[:],
                    )
                psv = psT[:].rearrange("p (m e) -> p m e", e=128)
                gl = psv[:, :, 0:E]       # gate logits [128, M, E] (PSUM)
                nl = psv[:, :, E:2 * E]   # noise logits [128, M, E] (PSUM)

                # softplus(x) = relu(x) + ln(1 + exp(-|x|))  (no Softplus ACT
                # table in this toolchain; exp/ln share one table set).
                # Batched across the whole strip to limit ACT table swaps.
                ab = spool.tile([128, M_TILES, E], F32, tag="ab")
                nc.scalar.activation(ab[:], nl, AF.Abs)
                ex = spool.tile([128, M_TILES, E], F32, tag="ex")
                nc.scalar.activation(ex[:], ab[:], AF.Exp, scale=-1.0)
                ln1 = spool.tile([128, M_TILES, E], F32, tag="ln1")
                nc.scalar.activation(ln1[:], ex[:], AF.Ln, bias=1.0)
                sp_t = spool.tile([128, M_TILES, E], F32, tag="sp")
                nc.vector.scalar_tensor_tensor(
                    sp_t[:], nl, 0.0, ln1[:], OP.max, OP.add
                )
                tmp = spool.tile([128, M_TILES, E], F32, tag="tmp")
                nc.vector.tensor_mul(tmp[:], nzt[:], sp_t[:])
                nc.vector.tensor_add(lgt[:], gl, tmp[:])

                mxa = spool.tile([128, M_TILES, 8], F32, tag="mxa")
                ixa = spool.tile([128, M_TILES, 8], U32, tag="ixa")
                for m in range(M_TILES):
                    nc.vector.max(mxa[:, m, :], lgt[:, m, :])
                    nc.vector.max_index(ixa[:, m, :], mxa[:, m, :], lgt[:, m, :])

                # top-2 softmax weights: w1 = 1/(1+exp(l2-l1)), w2 = 1-w1
                d = spool.tile([128, M_TILES], F32, tag="d")
                nc.vector.tensor_sub(d[:], mxa[:, :, 1], mxa[:, :, 0])
                e2 = spool.tile([128, M_TILES], F32, tag="e2")
                nc.scalar.activation(e2[:], d[:], AF.Exp)
                opd = spool.tile([128, M_TILES], F32, tag="opd")
                nc.vector.tensor_scalar_add(opd[:], e2[:], 1.0)
                w1 = spool.tile([128, M_TILES], F32, tag="w1")
                nc.vector.reciprocal(w1[:], opd[:])
                w2 = spool.tile([128, M_TILES], F32, tag="w2")
                nc.vector.tensor_mul(w2[:], e2[:], w1[:])

                for m in range(M_TILES):
                    lg = lgt[:, m, :]
                    eq1 = spool.tile([128, E], F32, tag="eq1")
                    nc.vector.tensor_scalar(
                        eq1[:], lg, mxa[:, m, 0:1], None, OP.is_equal
                    )
                    eq2 = spool.tile([128, E], F32, tag="eq2")
                    nc.vector.tensor_scalar(
                        eq2[:], lg, mxa[:, m, 1:2], None, OP.is_equal
                    )
                    t2 = spool.tile([128, E], F32, tag="t2")
                    nc.vector.tensor_scalar_mul(t2[:], eq2[:], w2[:, m:m + 1])
                    nc.vector.scalar_tensor_tensor(
                        gat[:, m, :], eq1[:], w1[:, m:m + 1], t2[:],
                        OP.mult, OP.add,
                    )
                nc.vector.tensor_copy(ixt[:], ixa[:, :, 0:2])

                nc.gpsimd.dma_start(out=logits_o[s], in_=lgt[:])
                nc.gpsimd.dma_start(out=gates_o[s], in_=gat[:])
                nc.gpsimd.dma_start(out=idx_o[s], in_=ixt[:])

    nc.compile()
    return nc


def kernel(hidden_states, noise, w_gate, w_noise, mm_dtype="fp32", trace=None):
    global LAST_RESULTS

    if trace is None:
        trace = bool(os.environ.get("KERNEL_TRACE"))
    mm_dt = {"fp32": F32, "fp32r": mybir.dt.float32r}[mm_dtype]
    # fp32r streams at full rate only with a moving dim >= 256 -> zero-pad.
    nw = 128 if mm_dtype == "fp32" else 256

    key = (mm_dtype, nw)
    if key not in _BUILD_CACHE:
        _BUILD_CACHE[key] = _build(mm_dt, nw)
    nc = _BUILD_CACHE[key]

    X = np.ascontiguousarray(np.asarray(hidden_states, dtype=np.float32)).reshape(T, H)
    nzf = np.ascontiguousarray(np.asarray(noise, dtype=np.float32)).reshape(T, E)
    wg = np.asarray(w_gate, dtype=np.float32)
    wn = np.asarray(w_noise, dtype=np.float32)

    wt_full = np.zeros((H, nw), dtype=np.float32)
    wt_full[:, :E] = wg.T
    wt_full[:, E:2 * E] = wn.T

    # xt stream order: [KG, STRIPS, 128p, KPG, U] from X^T [H, TC]
    # X^T[h, t] with h = (kg*KPG + kk)*128 + p, t = s*U + u.
    xt_all = (
        X.reshape(NCORES, TC, H)
        .transpose(0, 2, 1)                          # [c, H, TC]
        .reshape(NCORES, KG, KPG, 128, STRIPS, U)
        .transpose(0, 1, 4, 3, 2, 5)                 # [c, KG, STRIPS, p, KPG, U]
    )
    xt_all = np.ascontiguousarray(xt_all)
    # noise order: [STRIPS, 128p, M_TILES, E]; token t = s*U + m*128 + p
    nz_all = np.ascontiguousarray(
        nzf.reshape(NCORES, STRIPS, M_TILES, 128, E).transpose(0, 1, 3, 2, 4)
    )

    ident = np.eye(128, dtype=np.float32)
    in_maps = [
        {"xt": xt_all[c], "nz": nz_all[c], "wt": wt_full, "ident": ident}
        for c in range(NCORES)
    ]

    LAST_RESULTS = run_bass_kernel_spmd(
        nc, in_maps, list(range(NCORES)), trace=trace
    )
    results = LAST_RESULTS.results

    def unshuffle(key, dt):
        # [c][STRIPS, 128, M_TILES, X] -> token order t = s*U + m*128 + p
        a = np.stack([results[c][key] for c in range(NCORES)])
        a = a.transpose(0, 1, 3, 2, 4)  # [c, s, m, p, X]
        return np.ascontiguousarray(a).reshape(T, -1).astype(dt, copy=False)

    logits = unshuffle("logits_o", np.float32).reshape(B, S, E)
    gates = unshuffle("gates_o", np.float32).reshape(B, S, E)
    idx = unshuffle("idx_o", np.int32).reshape(B, S, 2)
    return gates, idx, logits
